# revision 1
# baseline (speedup 1.0000x reference)
"""Trainium2 Bass kernel for a 2-layer GCN (gnn_message_passing).

Reference computation (all f32 inputs):
    h      = relu(adj @ (x @ W1) + b1)        adj: [N, N], x: [N, F]
    logits = adj @ (h @ W2) + b2
    out    = log_softmax(logits, axis=1)       out: [N, C]

Distribution: 1-D row partition over 8 NeuronCores. Core t owns rows
R_t = [t*M, (t+1)*M). adj is symmetric (by construction), so the column
slice adj[:, R_t] in natural row-major layout is exactly the transposed
operand adj_t^T the TensorEngine needs as its moving operand — no
on-chip transpose of adj is ever required.

Per-core plan (single NEFF launch, two AllGathers, no warm-up dummy):
  - x^T is pre-transposed on the host, so S_t = x_t @ W1 needs no PE
    transposes; S is ready ~20us in and its AllGather is the FIRST
    collective — it absorbs the one-time ncfw setup itself (a separate
    dummy AllGather would only push the S gather later).
  - AllGather payloads use a [128, blk] block layout so both the
    bounce-buffer store and the gathered load move 2KB-contiguous
    lines (the naive [rows, feat] layout makes 256B-line descriptors).
  - adj[:, R_t] streams as f32 on three DMA paths (sync HWDGE, scalar
    HWDGE, gpsimd SWDGE) concurrently and is cast to bf16 into a
    resident 16MB SBUF tile; layer-1 matmuls consume chunks in cast
    order, layer 2 reuses the residency with zero extra HBM traffic.
  - log_softmax is split per layer-2 half so the reductions/exponents
    overlap the other half's matmuls; EXP/LN tables preloaded early.
  - Output leaves in a per-partition-contiguous [128, MC*C] layout
    (single fat DMA); the host reorders rows.

kernel(**inputs) takes FULL inputs and returns the FULL [N, C] output.
"""

import numpy as np

import concourse.bass as bass
import concourse.mybir as mybir
import concourse.tile as tile
from concourse import bacc
from concourse.bass_utils import run_bass_kernel_spmd
from concourse.masks import make_identity

NCORES = 8
N_FULL = 8192
NFEAT = 512
NHID = 128
NCLASS = 40
F32 = mybir.dt.float32
BF16 = mybir.dt.bfloat16


def build(n_total: int = N_FULL):
    """Build the SPMD Bass graph for one core (same program on all 8)."""
    M = n_total // NCORES          # rows owned by this core
    K = n_total // 128             # 128-row contraction chunks
    MC = M // 128                  # 128-row output chunks on this core
    A = K // NCORES                # contraction chunks per core block (8)
    DF = NFEAT // 128              # feature chunks (4)
    KK = 2 if K % 2 == 0 else 1    # adj k-chunks per DMA superchunk
    MW = min(512, M)               # free-dim split width
    MH = M // MW

    nc = bacc.Bacc(
        "TRN2", target_bir_lowering=False, debug=False,
        enable_asserts=True, num_devices=NCORES,
    )

    xiT = nc.dram_tensor("xiT", [NFEAT, M], F32, kind="ExternalInput")
    adjc = nc.dram_tensor("adjc", [n_total, M], F32, kind="ExternalInput")
    w1 = nc.dram_tensor("w1", [NFEAT, NHID], F32, kind="ExternalInput")
    b1 = nc.dram_tensor("b1", [NHID, 1], F32, kind="ExternalInput")
    w2 = nc.dram_tensor("w2", [NHID, NCLASS], F32, kind="ExternalInput")
    b2 = nc.dram_tensor("b2", [NCLASS, 1], F32, kind="ExternalInput")
    # out[p, a*NCLASS + c] = log_softmax of row a*128+p; host re-interleaves.
    out_ext = nc.dram_tensor("out", [128, MC * NCLASS], F32, kind="ExternalOutput")

    rg = [list(range(NCORES))]

    with tile.TileContext(nc) as tc:
        with (
            tc.tile_pool(name="resident", bufs=1) as res,
            tc.tile_pool(name="dram", bufs=1, space="DRAM") as dram,
        ):
            adjres = res.tile([128, K * M], BF16)          # adj_t^T, bf16, resident
            sres = res.tile([128, K, NHID], BF16)          # gathered S, chunk layout
            zres = res.tile([128, K, NCLASS], BF16)        # gathered z, chunk layout
            hT = res.tile([128, M], BF16)                  # layer-1 out, [h, m]
            w1st = res.tile([128, DF, NHID], F32)
            w2bf = res.tile([128, NCLASS], BF16)
            b1sb = res.tile([128, 1], F32)
            b2sb = res.tile([NCLASS, 1], F32)
            ident = res.tile([128, 128], F32)
            sloc = res.tile([128, A, NHID], BF16)          # local S block
            zloc = res.tile([128, A, NCLASS], BF16)        # local z block
            lTsb = res.tile([NCLASS, M], F32)              # logits^T (+b2)
            osb = res.tile([128, MC, NCLASS], F32)         # final log-softmax out
            scr = res.tile([128, 1], F32)                  # act-table warmup scratch

            # collective bounce buffers (internal DRAM). Block layout:
            # s_in[p, a*NHID+f] = S_t[a*128+p, f]; the AllGather stacks the
            # 8 cores along dim0, so global chunk k = j*A + a lands at
            # s_out[j*128+p, a, f] -> "(j p) a f -> p (j a) f" reads are
            # 2KB-contiguous per (p, j).
            s_in = dram.tile([128, A * NHID], BF16)
            s_out = dram.tile([NCORES * 128, A * NHID], BF16, addr_space="Shared")
            z_in = dram.tile([128, A * NCLASS], BF16)
            z_out = dram.tile([NCORES * 128, A * NCLASS], BF16, addr_space="Shared")

            # ---- adj staging pools: allocated FIRST so their SBUF zones
            # never overlap the (short-lived) consts pool — an overlap makes
            # the first adj DMA wait for the whole S phase to release
            # memory. One pool per DMA path, so a slow path never blocks
            # buffer recycling for the fast ones. ----
            astage_s_ctx = tc.tile_pool(name="astage_s", bufs=2)
            astage_s = astage_s_ctx.__enter__()
            astage_c_ctx = tc.tile_pool(name="astage_c", bufs=2)
            astage_c = astage_c_ctx.__enter__()
            astage_g_ctx = tc.tile_pool(name="astage_g", bufs=2)
            astage_g = astage_g_ctx.__enter__()

            # ---- constants + x^T. x^T halves ride the two HWDGE rings
            # (ahead of the adj chunks in each ring's queue); everything
            # else uses the gpsimd SWDGE queue. ----
            make_identity(nc, ident[:, :])
            with tc.tile_pool(name="consts", bufs=1) as cst:
                nc.gpsimd.dma_start(
                    out=w1st[:, :, :],
                    in_=w1.ap().rearrange("(a p) f -> p a f", p=128),
                )
                DH = DF // 2
                xst = cst.tile([128, DH, M], F32)
                xst2 = cst.tile([128, DH, M], F32)
                nc.sync.dma_start(
                    out=xst[:, :, :],
                    in_=xiT.ap().rearrange("(d p) m -> p d m", p=128)[:, 0:DH, :],
                )
                nc.scalar.dma_start(
                    out=xst2[:, :, :],
                    in_=xiT.ap().rearrange("(d p) m -> p d m", p=128)[:, DH:DF, :],
                )

                w2st = cst.tile([128, NCLASS], F32)
                nc.gpsimd.dma_start(out=w2st[:, :], in_=w2.ap())
                nc.gpsimd.tensor_copy(w2bf[:, :], w2st[:, :])
                nc.gpsimd.dma_start(out=b1sb[:, :], in_=b1.ap())
                nc.gpsimd.dma_start(out=b2sb[:, :], in_=b2.ap())

                # Pre-load the EXP and LN activation tables into their two
                # table slots so the softmax tail skips the ~1.3us
                # ACT_TABLE_LOADs.
                nc.scalar.activation(
                    scr[:, :], b1sb[:, :], mybir.ActivationFunctionType.Exp
                )
                nc.scalar.activation(
                    scr[:, :], scr[:, :], mybir.ActivationFunctionType.Ln
                )

                # ---- S phase: S_t = x_t @ W1 (no transposes needed) ----
                with tc.tile_pool(name="spsum", bufs=2, space="PSUM") as spsum:
                    for nci in range(A):
                        ps = spsum.tile([128, NHID], F32, tag="ps")
                        for d in range(DF):
                            xsrc = xst if d < DH else xst2
                            nc.tensor.matmul(
                                ps[:, :],
                                xsrc[:, d % DH, nci * 128:(nci + 1) * 128],
                                w1st[:, d, :],
                                start=(d == 0), stop=(d == DF - 1),
                            )
                        nc.vector.tensor_copy(sloc[:, nci, :], ps[:, :])
                nc.gpsimd.dma_start(
                    out=s_in.rearrange("p (a f) -> p a f", a=A), in_=sloc[:, :, :]
                )

            # ---- S AllGather: the FIRST collective — absorbs ncfw setup ----
            nc.gpsimd.collective_compute(
                "AllGather", mybir.AluOpType.bypass, replica_groups=rg,
                ins=[s_in[:, :]], outs=[s_out[:, :]],
            )

            # ---- adj stream: f32 HBM -> bf16 resident SBUF, alternating
            # between the two HWDGE rings. The gathered-S load is sliced
            # into 8 small pieces interleaved into the ring queues at the
            # position each ring reaches around the time the S AllGather
            # lands — a single post-AG load would head-of-line block a
            # ring mid-stream until the collective finishes. ----
            sres4 = sres.rearrange("p (j a) f -> p j a f", j=NCORES)
            s_out4 = s_out.rearrange("(j p) (a f) -> p j a f", p=128, a=A)
            # SWDGE path gets the TAIL chunks: the ncfw AllGather starves
            # both HWDGE rings for its whole execution window (~80-100us),
            # but SWDGE keeps streaming through it.
            GSET = set(range(40, 64, 2))       # 12 chunks on the SWDGE path
            nring = 0
            for k in range(K):
                if k in GSET:
                    dma_eng, pool = nc.gpsimd, astage_g
                else:
                    if nring % 2 == 0:
                        dma_eng, pool = nc.sync, astage_s
                    else:
                        dma_eng, pool = nc.scalar, astage_c
                    nring += 1
                ast = pool.tile([128, M], F32, tag="ast")
                dma_eng.dma_start(
                    out=ast[:, :],
                    in_=adjc[k * 128:(k + 1) * 128, :],
                )
                nc.vector.tensor_copy(adjres[:, k * M:(k + 1) * M], ast[:, :])
            # gathered-S load: 8 pieces on the gpsimd SWDGE queue, which
            # drains its adj share right around the time the S AllGather
            # lands (~105us). On a ring this load head-of-line blocks adj
            # chunks the scheduler queues behind it (its sim models the
            # collective as fast); on SWDGE it overlaps the ring tails and
            # still completes well before the last casts gate layer 1.
            for j in range(NCORES):
                nc.gpsimd.dma_start(out=sres4[:, j, :, :], in_=s_out4[:, j, :, :])

            # ---- layer 1: hT += S_k^T @ adjT_k (pipelines with the cast
            # stream; starts when the S gather lands) ----
            with tc.tile_pool(name="hpsum", bufs=1, space="PSUM") as hp:
                ph = [hp.tile([128, MW], F32, name=f"ph{m}") for m in range(MH)]
                for k in range(K):
                    for mh in range(MH):
                        nc.tensor.matmul(
                            ph[mh][:, :],
                            sres[:, k, :],
                            adjres[:, k * M + mh * MW:k * M + (mh + 1) * MW],
                            start=(k == 0), stop=(k == K - 1),
                        )
                for mh in range(MH):
                    nc.scalar.activation(
                        hT[:, mh * MW:(mh + 1) * MW], ph[mh][:, :],
                        mybir.ActivationFunctionType.Relu,
                        bias=b1sb[:, 0:1], scale=1.0,
                    )
            astage_g_ctx.__exit__(None, None, None)
            astage_c_ctx.__exit__(None, None, None)
            astage_s_ctx.__exit__(None, None, None)

            # ---- z_t = h_t @ W2 ----
            with tc.tile_pool(name="zpsum", bufs=2, space="PSUM") as zp:
                for mc in range(MC):
                    pz = zp.tile([128, NCLASS], F32, tag="pz")
                    nc.tensor.matmul(
                        pz[:, :],
                        hT[:, mc * 128:(mc + 1) * 128],
                        w2bf[:, :],
                        start=True, stop=True,
                    )
                    nc.vector.tensor_copy(zloc[:, mc, :], pz[:, :])
            nc.gpsimd.dma_start(
                out=z_in.rearrange("p (a c) -> p a c", a=A), in_=zloc[:, :, :]
            )
            nc.gpsimd.collective_compute(
                "AllGather", mybir.AluOpType.bypass, replica_groups=rg,
                ins=[z_in[:, :]], outs=[z_out[:, :]],
            )

            nc.sync.dma_start(
                out=zres.rearrange("p (j a) c -> p j a c", j=NCORES)[:, 0:NCORES // 2, :, :],
                in_=z_out.rearrange("(j p) (a c) -> p j a c", p=128, a=A)[:, 0:NCORES // 2, :, :],
            )
            nc.scalar.dma_start(
                out=zres.rearrange("p (j a) c -> p j a c", j=NCORES)[:, NCORES // 2:, :, :],
                in_=z_out.rearrange("(j p) (a c) -> p j a c", p=128, a=A)[:, NCORES // 2:, :, :],
            )

            # ---- layer 2 + log_softmax, split into MH free-dim halves so
            # the vector/scalar softmax work overlaps the other half's
            # matmuls. adj is resident -> zero extra HBM traffic. ----
            with (
                tc.tile_pool(name="lpsum", bufs=1, space="PSUM") as lp,
                tc.tile_pool(name="smp", bufs=1, space="PSUM") as smp,
                tc.tile_pool(name="sms", bufs=1) as sms,
            ):
                ptrs = smp.tile([128, MC, NCLASS], F32)
                mx = sms.tile([128, MC], F32)
                ssum = sms.tile([128, MC], F32)
                lse = sms.tile([128, MC], F32)
                bias2 = sms.tile([128, MC], F32)
                esc = sms.tile([128, MC, NCLASS], F32)
                MCH = MC // MH                 # 128-row chunks per half
                for mh in range(MH):
                    # Column-tiled: the stationary z chunk is only 40 wide,
                    # so even-k and odd-k accumulations run CONCURRENTLY on
                    # disjoint 64-column groups of the PE array (~2x).
                    pl = lp.tile([128, MW], F32, tag="pl")
                    for k in range(0, K, 2):
                        nc.tensor.matmul(
                            pl[0:NCLASS, :],
                            zres[:, k, :],
                            adjres[:, k * M + mh * MW:k * M + (mh + 1) * MW],
                            start=(k == 0), stop=(k == K - 2),
                            tile_position=(0, 0),
                        )
                        nc.tensor.matmul(
                            pl[64:64 + NCLASS, :],
                            zres[:, k + 1, :],
                            adjres[:, (k + 1) * M + mh * MW:(k + 1) * M + (mh + 1) * MW],
                            start=(k == 0), stop=(k == K - 2),
                            tile_position=(0, 64),
                        )
                    nc.scalar.activation(
                        lTsb[:, mh * MW:(mh + 1) * MW], pl[64:64 + NCLASS, :],
                        mybir.ActivationFunctionType.Identity,
                        bias=b2sb[:, 0:1], scale=1.0,
                    )
                    nc.vector.tensor_tensor(
                        lTsb[:, mh * MW:(mh + 1) * MW],
                        lTsb[:, mh * MW:(mh + 1) * MW], pl[0:NCLASS, :],
                        op=mybir.AluOpType.add,
                    )
                    for mc in range(mh * MCH, (mh + 1) * MCH):
                        nc.tensor.transpose(
                            ptrs[:, mc, :], lTsb[:, mc * 128:(mc + 1) * 128],
                            ident[0:NCLASS, 0:NCLASS],
                        )
                        nc.vector.tensor_reduce(
                            mx[:, mc:mc + 1], ptrs[:, mc, :],
                            axis=mybir.AxisListType.X,
                            op=mybir.AluOpType.max, negate=True,
                        )
                        nc.scalar.activation(
                            esc[:, mc, :], ptrs[:, mc, :],
                            mybir.ActivationFunctionType.Exp,
                            bias=mx[:, mc:mc + 1], scale=1.0,
                            accum_out=ssum[:, mc:mc + 1],
                        )
                nc.scalar.activation(
                    lse[:, :], ssum[:, :], mybir.ActivationFunctionType.Ln,
                )
                nc.vector.tensor_sub(bias2[:, :], mx[:, :], lse[:, :])
                for mc in range(MC):
                    nc.scalar.activation(
                        osb[:, mc, :], ptrs[:, mc, :],
                        mybir.ActivationFunctionType.Identity,
                        bias=bias2[:, mc:mc + 1], scale=1.0,
                    )
            # contiguous per-partition lines; host reorders. Split in two so
            # the first half overlaps the second half's epilogue.
            oview = out_ext.ap().rearrange("p (a c) -> p a c", a=MC)
            nc.sync.dma_start(
                out=oview[:, 0:MC // 2, :], in_=osb[:, 0:MC // 2, :]
            )
            nc.sync.dma_start(
                out=oview[:, MC // 2:, :], in_=osb[:, MC // 2:, :]
            )

    nc.compile()
    return nc


_NC_CACHE = {}


def _get_nc(n_total: int):
    if n_total not in _NC_CACHE:
        _NC_CACHE[n_total] = build(n_total)
    return _NC_CACHE[n_total]


def make_in_maps(x, adj, W1, b1, W2, b2):
    n_total = x.shape[0]
    m = n_total // NCORES
    in_maps = []
    for t in range(NCORES):
        c0 = t * m
        in_maps.append({
            "xiT": np.ascontiguousarray(x[c0:c0 + m].T),
            "adjc": np.ascontiguousarray(adj[:, c0:c0 + m]),
            "w1": np.ascontiguousarray(W1),
            "b1": np.ascontiguousarray(b1.reshape(NHID, 1)),
            "w2": np.ascontiguousarray(W2),
            "b2": np.ascontiguousarray(b2.reshape(NCLASS, 1)),
        })
    return in_maps


def _assemble(res_list):
    """[128, MC*NCLASS] per core -> [N, NCLASS]."""
    outs = []
    for r in res_list:
        o = r["out"]
        mc = o.shape[1] // NCLASS
        outs.append(
            o.reshape(128, mc, NCLASS).transpose(1, 0, 2).reshape(-1, NCLASS)
        )
    return np.concatenate(outs, axis=0)


def kernel(x, adj, W1, b1, W2, b2):
    x = np.asarray(x, dtype=np.float32)
    adj = np.asarray(adj, dtype=np.float32)
    W1 = np.asarray(W1, dtype=np.float32)
    b1 = np.asarray(b1, dtype=np.float32)
    W2 = np.asarray(W2, dtype=np.float32)
    b2 = np.asarray(b2, dtype=np.float32)
    nc = _get_nc(x.shape[0])
    in_maps = make_in_maps(x, adj, W1, b1, W2, b2)
    res = run_bass_kernel_spmd(nc, in_maps, list(range(NCORES)))
    return _assemble([res.results[i] for i in range(NCORES)])



# revision 10
# speedup vs baseline: 1.1011x; 1.1011x over previous
"""Trainium2 Bass kernel for a 2-layer GCN (gnn_message_passing).

Reference computation (all f32 inputs):
    h      = relu(adj @ (x @ W1) + b1)        adj: [N, N], x: [N, F]
    logits = adj @ (h @ W2) + b2
    out    = log_softmax(logits, axis=1)       out: [N, C]

Distribution: 1-D row partition over 8 NeuronCores. Core t owns rows
R_t = [t*M, (t+1)*M). adj is symmetric, so adj[:, R_t] (shipped in
natural column-slice layout) doubles as the transposed moving operand
for both layers.

v3 design (vs the f32/bf16 + double-AllGather baseline):
  - adj ships from the HOST as fp8e4m3 in a paired-chunk layout
    [128, K2, 2, M] (8 MB/core instead of 32): DMA lands directly in
    the resident SBUF tile - no staging pools, no on-chip casts. The
    fp8 pairing feeds layer-2's DoubleRow matmuls (0.5 cyc/row) and
    layer-1 streams it as a plain fp8 moving operand (1 cyc/row).
  - the S AllGather is gone: every core computes the FULL S^T = W1^T x
    locally from a replicated bf16 x (fat 512-wide matmuls, 4 ldw per
    512 columns), then converts S^T -> S chunks with DMA-xbar
    transposes (off the PE). S-in-fp8 fails the accuracy budget
    (2.5e-2 > 2e-2 measured), so S stays bf16 and layer-1 runs mixed
    fp8 x bf16.
  - a tiny dummy AllGather triggers at t~0 so the one-time ncfw setup
    barrier (~36us) runs concurrent with the input streams.
  - z = h@W2 is scaled by 1/16 and cast to fp8 for a small AllGather
    (40 KB in / 320 KB out); the x16 rides the layer-2 epilogue
    activation (out = 16*psum + b2). Layer-2 accumulates two adj chunk
    pairs concurrently via PE column groups (tile_position 0 / 64).
  - log_softmax tail identical to the baseline (PE transposes of
    logits^T chunks + DVE/ACT reductions), EXP/LN tables preloaded.

kernel(**inputs) takes FULL inputs and returns the FULL [N, C] output.
"""

import numpy as np
import ml_dtypes

import concourse.bass as bass
import concourse.mybir as mybir
import concourse.tile as tile
from concourse import bacc
from concourse.bass_utils import run_bass_kernel_spmd
from concourse.masks import make_identity

NCORES = 8
N_FULL = 8192
NFEAT = 512
NHID = 128
NCLASS = 40
F32 = mybir.dt.float32
BF16 = mybir.dt.bfloat16
FP8 = mybir.dt.float8e4
ZSCALE = 16.0


def build(n_total: int = N_FULL):
    """Build the SPMD Bass graph for one core (same program on all 8)."""
    M = n_total // NCORES          # rows owned by this core
    K = n_total // 128             # 128-row contraction chunks (64)
    K2 = K // 2                    # fp8 DoubleRow chunk pairs (32)
    G = n_total // 512             # 512-node S^T column groups (16)
    MC = M // 128                  # 128-row output chunks on this core (8)
    A = M // 128                   # local 128-node chunks (8)
    DF = NFEAT // 128              # feature chunks (4)
    MW = min(512, M)               # free-dim split width
    MH = M // MW                   # halves of the local rows (2)

    nc = bacc.Bacc(
        "TRN2", target_bir_lowering=False, debug=False,
        enable_asserts=True, num_devices=NCORES,
    )

    # xb[p, g*DF*512 + d*512 + m'] = x[g*512+m', d*128+p]   (replicated)
    xb = nc.dram_tensor("xb", [128, G * DF * 512], BF16, kind="ExternalInput")
    # adjp[p, ((k2*2+i)*M) + m] = adj[(k2*2+i)*128+p, c0+m]  fp8
    adjp = nc.dram_tensor("adjp", [128, K * M], FP8, kind="ExternalInput")
    w1 = nc.dram_tensor("w1", [NFEAT, NHID], F32, kind="ExternalInput")
    b1 = nc.dram_tensor("b1", [NHID, 1], F32, kind="ExternalInput")
    w2 = nc.dram_tensor("w2", [NHID, NCLASS], F32, kind="ExternalInput")
    b2 = nc.dram_tensor("b2", [NCLASS, 1], F32, kind="ExternalInput")
    # out[p, a*NCLASS + c] = log_softmax of row a*128+p; host re-interleaves.
    out_ext = nc.dram_tensor("out", [128, MC * NCLASS], F32, kind="ExternalOutput")

    rg = [list(range(NCORES))]

    with tile.TileContext(nc) as tc:
        with (
            tc.tile_pool(name="resident", bufs=1) as res,
            tc.tile_pool(name="dram", bufs=1, space="DRAM") as dram,
        ):
            adjres = res.tile([128, K2, 2, M], FP8)        # adj_t^T resident
            xst = res.tile([128, G, DF, 512], BF16)        # replicated x
            sTt = res.tile([128, n_total], BF16)           # S^T = (x@W1)^T
            sres = res.tile([128, K, NHID], BF16)          # S chunks [node,k,hid]
            hTt = res.tile([128, M], BF16)                 # layer-1 out, [hid, m]
            zres = res.tile([128, K2, 2, NCLASS], FP8)     # gathered z/16, fp8
            zloc = res.tile([128, A, NCLASS], FP8)         # local z/16, fp8
            w1bf = res.tile([128, DF, NHID], BF16)
            w2bf = res.tile([128, NCLASS], BF16)
            b1sb = res.tile([128, 1], F32)
            b2sb = res.tile([NCLASS, 1], F32)
            ident = res.tile([128, 128], F32)
            lTsb = res.tile([NCLASS, M], F32)              # logits^T (+b2)
            osb = res.tile([128, MC, NCLASS], F32)         # final log-softmax out
            scr = res.tile([128, 1], F32)                  # act-table warmup
            dum = res.tile([1, 16], BF16)                  # dummy-AG payload

            # collective bounce buffers (internal DRAM).
            d_in = dram.tile([1, 16], BF16)
            d_out = dram.tile([NCORES, 16], BF16, addr_space="Shared")
            z_in = dram.tile([128, A * NCLASS], FP8)
            z_out = dram.tile([NCORES * 128, A * NCLASS], FP8, addr_space="Shared")

            # ---- dummy AllGather: very first gpsimd work, so the one-time
            # ncfw setup barrier runs while the x/adj streams warm up. ----
            nc.gpsimd.memset(dum[:, :], 0.0)
            nc.gpsimd.dma_start(out=d_in[:, :], in_=dum[:, :])
            nc.gpsimd.collective_compute(
                "AllGather", mybir.AluOpType.bypass, replica_groups=rg,
                ins=[d_in[:, :]], outs=[d_out[:, :]],
            )

            make_identity(nc, ident[:, :])

            # ---- consts on the gpsimd SWDGE queue ----
            with tc.tile_pool(name="consts", bufs=1) as cst:
                w1st = cst.tile([128, DF, NHID], F32)
                w2st = cst.tile([128, NCLASS], F32)
                nc.gpsimd.dma_start(
                    out=w1st[:, :, :],
                    in_=w1.ap().rearrange("(a p) f -> p a f", p=128),
                )
                nc.gpsimd.tensor_copy(w1bf[:, :, :], w1st[:, :, :])
                nc.gpsimd.dma_start(out=w2st[:, :], in_=w2.ap())
                nc.gpsimd.tensor_copy(w2bf[:, :], w2st[:, :])
                nc.gpsimd.dma_start(out=b1sb[:, :], in_=b1.ap())
                nc.gpsimd.dma_start(out=b2sb[:, :], in_=b2.ap())

                # EXP/LN activation tables into their slots before the tail.
                nc.scalar.activation(
                    scr[:, :], b1sb[:, :], mybir.ActivationFunctionType.Exp
                )
                nc.scalar.activation(
                    scr[:, :], scr[:, :], mybir.ActivationFunctionType.Ln
                )

                # ---- early adj chunks on the SWDGE queue (layer-1 head) ----
                NG_ADJ = 6
                adjv = adjp.ap().rearrange("p (k i m) -> p k i m", k=K2, i=2)
                for k2 in range(NG_ADJ):
                    nc.gpsimd.dma_start(
                        out=adjres[:, k2, :, :], in_=adjv[:, k2, :, :]
                    )

                # ---- x stream: first half, alternating rings ----
                xv = xb.ap().rearrange("p (g d m) -> p g d m", g=G, d=DF)
                for g in range(0, 8):
                    eng = nc.sync if g % 2 == 0 else nc.scalar
                    eng.dma_start(out=xst[:, g, :, :], in_=xv[:, g, :, :])

                # ---- S^T phase (interleaved with layer-1 chunks so the PE
                # fills x-stream stalls with adj work), plus xbar transposes
                # and the second half of the x stream. ----
                sres3 = sres.rearrange("p k f -> p (k f)")
                with (
                    tc.tile_pool(name="spsum", bufs=2, space="PSUM") as spsum,
                    tc.tile_pool(name="hpsum", bufs=1, space="PSUM") as hp,
                ):
                    ph = [hp.tile([128, MW], F32, name=f"ph{m}") for m in range(MH)]

                    def l1_chunk(k):
                        k2, i = divmod(k, 2)
                        for mh in range(MH):
                            nc.tensor.matmul(
                                ph[mh][:, :],
                                sres[:, k, :],
                                adjres[:, k2, i, mh * MW:(mh + 1) * MW],
                                start=(k == 0), stop=(k == K - 1),
                            )

                    for g in range(G):
                        ps = spsum.tile([128, 512], F32, tag="ps")
                        for d in range(DF):
                            nc.tensor.matmul(
                                ps[:, :],
                                w1bf[:, d, :],
                                xst[:, g, d, :],
                                start=(d == 0), stop=(d == DF - 1),
                            )
                        nc.vector.tensor_copy(
                            sTt[:, g * 512:(g + 1) * 512], ps[:, :]
                        )
                        # S^T -> S chunks through the DMA transpose xbar
                        eng = nc.sync if g % 2 == 0 else nc.scalar
                        eng.dma_start(
                            out=sres3[:, g * 4 * NHID:(g + 1) * 4 * NHID]
                            .rearrange("p (j f) -> p j f", j=4),
                            in_=sTt[:, g * 512:(g + 1) * 512],
                            transpose=True,
                        )
                        if g + 8 < G:
                            eng2 = nc.sync if g % 2 == 0 else nc.scalar
                            eng2.dma_start(
                                out=xst[:, g + 8, :, :], in_=xv[:, g + 8, :, :]
                            )
                        # remaining adj chunks, two per iteration, issued
                        # ahead of the layer-1 chunks that consume them
                        for k2 in (2 * g, 2 * g + 1):
                            if NG_ADJ <= k2 < K2:
                                eng3 = nc.sync if k2 % 2 == 0 else nc.scalar
                                eng3.dma_start(
                                    out=adjres[:, k2, :, :],
                                    in_=adjv[:, k2, :, :],
                                )
                        if g >= 1:
                            for k in range(4 * (g - 1), 4 * g):
                                l1_chunk(k)

                    for k in range(4 * (G - 1), K):
                        l1_chunk(k)

                    for mh in range(MH):
                        nc.scalar.activation(
                            hTt[:, mh * MW:(mh + 1) * MW], ph[mh][:, :],
                            mybir.ActivationFunctionType.Relu,
                            bias=b1sb[:, 0:1], scale=1.0,
                        )

            # ---- z_t = (h_t @ W2)/16 as fp8, AllGather ----
            with tc.tile_pool(name="zpsum", bufs=2, space="PSUM") as zp:
                for a in range(A):
                    pz = zp.tile([128, NCLASS], F32, tag="pz")
                    nc.tensor.matmul(
                        pz[:, :],
                        hTt[:, a * 128:(a + 1) * 128],
                        w2bf[:, :],
                        start=True, stop=True,
                    )
                    nc.scalar.activation(
                        zloc[:, a, :], pz[:, :],
                        mybir.ActivationFunctionType.Copy,
                        bias=0.0, scale=1.0 / ZSCALE,
                    )
            nc.gpsimd.dma_start(
                out=z_in.rearrange("p (a c) -> p a c", a=A), in_=zloc[:, :, :]
            )
            nc.gpsimd.collective_compute(
                "AllGather", mybir.AluOpType.bypass, replica_groups=rg,
                ins=[z_in[:, :]], outs=[z_out[:, :]],
            )

            zrf = zres.rearrange("p k i c -> p (k i) c")
            zov = z_out.rearrange("(j p) (a c) -> p j a c", p=128, a=A)
            nc.sync.dma_start(
                out=zrf[:, 0:K // 2, :].rearrange("p (j a) c -> p j a c", a=A),
                in_=zov[:, 0:NCORES // 2, :, :],
            )
            nc.scalar.dma_start(
                out=zrf[:, K // 2:, :].rearrange("p (j a) c -> p j a c", a=A),
                in_=zov[:, NCORES // 2:, :, :],
            )

            # ---- layer 2 (fp8 DoubleRow, 2 concurrent column groups) +
            # log_softmax, split per mh half so the vector/scalar softmax
            # work overlaps the other half's matmuls. ----
            with (
                tc.tile_pool(name="lpsum", bufs=1, space="PSUM") as lp,
                tc.tile_pool(name="smp", bufs=1, space="PSUM") as smp,
                tc.tile_pool(name="sms", bufs=1) as sms,
            ):
                ptrs = smp.tile([128, MC, NCLASS], F32)
                lttmp = sms.tile([NCLASS, MW], F32)
                mx = sms.tile([128, MC], F32)
                ssum = sms.tile([128, MC], F32)
                lse = sms.tile([128, MC], F32)
                bias2 = sms.tile([128, MC], F32)
                esc = sms.tile([128, MC, NCLASS], F32)
                MCH = MC // MH                 # 128-row chunks per half
                for mh in range(MH):
                    pl = lp.tile([128, MW], F32, tag="pl")
                    for k2 in range(K2):
                        # fp8 moving, two concurrent PE column groups
                        nc.tensor.matmul(
                            pl[0:NCLASS, :],
                            zres[:, k2, 0, :],
                            adjres[:, k2, 0, mh * MW:(mh + 1) * MW],
                            start=(k2 == 0), stop=(k2 == K2 - 1),
                            tile_position=(0, 0),
                            skip_group_check=True,
                        )
                        nc.tensor.matmul(
                            pl[64:64 + NCLASS, :],
                            zres[:, k2, 1, :],
                            adjres[:, k2, 1, mh * MW:(mh + 1) * MW],
                            start=(k2 == 0), stop=(k2 == K2 - 1),
                            tile_position=(0, 64),
                            skip_group_check=True,
                        )
                    nc.scalar.activation(
                        lTsb[:, mh * MW:(mh + 1) * MW], pl[64:64 + NCLASS, :],
                        mybir.ActivationFunctionType.Identity,
                        bias=b2sb[:, 0:1], scale=ZSCALE,
                    )
                    nc.scalar.activation(
                        lttmp[:, :], pl[0:NCLASS, :],
                        mybir.ActivationFunctionType.Copy,
                        bias=0.0, scale=ZSCALE,
                    )
                    nc.vector.tensor_tensor(
                        lTsb[:, mh * MW:(mh + 1) * MW],
                        lTsb[:, mh * MW:(mh + 1) * MW], lttmp[:, :],
                        op=mybir.AluOpType.add,
                    )
                    for mc in range(mh * MCH, (mh + 1) * MCH):
                        nc.tensor.transpose(
                            ptrs[:, mc, :], lTsb[:, mc * 128:(mc + 1) * 128],
                            ident[0:NCLASS, 0:NCLASS],
                        )
                        nc.vector.tensor_reduce(
                            mx[:, mc:mc + 1], ptrs[:, mc, :],
                            axis=mybir.AxisListType.X,
                            op=mybir.AluOpType.max, negate=True,
                        )
                        nc.scalar.activation(
                            esc[:, mc, :], ptrs[:, mc, :],
                            mybir.ActivationFunctionType.Exp,
                            bias=mx[:, mc:mc + 1], scale=1.0,
                            accum_out=ssum[:, mc:mc + 1],
                        )
                nc.scalar.activation(
                    lse[:, :], ssum[:, :], mybir.ActivationFunctionType.Ln,
                )
                nc.vector.tensor_sub(bias2[:, :], mx[:, :], lse[:, :])
                for mc in range(MC):
                    nc.scalar.activation(
                        osb[:, mc, :], ptrs[:, mc, :],
                        mybir.ActivationFunctionType.Identity,
                        bias=bias2[:, mc:mc + 1], scale=1.0,
                    )
            # contiguous per-partition lines; host reorders. Split in two so
            # the first half overlaps the second half's epilogue.
            oview = out_ext.ap().rearrange("p (a c) -> p a c", a=MC)
            nc.sync.dma_start(
                out=oview[:, 0:MC // 2, :], in_=osb[:, 0:MC // 2, :]
            )
            nc.sync.dma_start(
                out=oview[:, MC // 2:, :], in_=osb[:, MC // 2:, :]
            )

    nc.compile()
    return nc


_NC_CACHE = {}


def _get_nc(n_total: int):
    if n_total not in _NC_CACHE:
        _NC_CACHE[n_total] = build(n_total)
    return _NC_CACHE[n_total]


def make_in_maps(x, adj, W1, b1, W2, b2):
    n_total = x.shape[0]
    m = n_total // NCORES
    g = n_total // 512
    k2 = n_total // 256
    # xb[p, g, d, m'] = x^T[d*128+p, g*512+m']  (replicated, bf16)
    xT = np.ascontiguousarray(x.T.astype(ml_dtypes.bfloat16))
    xbp = np.ascontiguousarray(
        xT.reshape(DFG := NFEAT // 128, 128, g, 512).transpose(1, 2, 0, 3)
    ).reshape(128, g * DFG * 512)
    in_maps = []
    for t in range(NCORES):
        c0 = t * m
        # adjp[p, k2, i, m] = adj[(k2*2+i)*128+p, c0+m]  fp8
        asl = adj[:, c0:c0 + m].astype(ml_dtypes.float8_e4m3)
        ap8 = np.ascontiguousarray(
            asl.reshape(k2, 2, 128, m).transpose(2, 0, 1, 3)
        ).reshape(128, k2 * 2 * m)
        in_maps.append({
            "xb": xbp,
            "adjp": ap8,
            "w1": np.ascontiguousarray(W1),
            "b1": np.ascontiguousarray(b1.reshape(NHID, 1)),
            "w2": np.ascontiguousarray(W2),
            "b2": np.ascontiguousarray(b2.reshape(NCLASS, 1)),
        })
    return in_maps


def _assemble(res_list):
    """[128, MC*NCLASS] per core -> [N, NCLASS]."""
    outs = []
    for r in res_list:
        o = np.asarray(r["out"])
        mc = o.shape[1] // NCLASS
        outs.append(
            o.reshape(128, mc, NCLASS).transpose(1, 0, 2).reshape(-1, NCLASS)
        )
    return np.concatenate(outs, axis=0)


def kernel(x, adj, W1, b1, W2, b2):
    x = np.asarray(x, dtype=np.float32)
    adj = np.asarray(adj, dtype=np.float32)
    W1 = np.asarray(W1, dtype=np.float32)
    b1 = np.asarray(b1, dtype=np.float32)
    W2 = np.asarray(W2, dtype=np.float32)
    b2 = np.asarray(b2, dtype=np.float32)
    nc = _get_nc(x.shape[0])
    in_maps = make_in_maps(x, adj, W1, b1, W2, b2)
    res = run_bass_kernel_spmd(nc, in_maps, list(range(NCORES)))
    return _assemble([res.results[i] for i in range(NCORES)])
